# revision 8
# baseline (speedup 1.0000x reference)
"""AURC loss kernel for Trainium2, sharded across 8 NeuronCores.

Algorithm (matches the reference):
  logp = log_softmax(x);  score = exp(max logp);  loss = -logp[target]
  sort by score desc;  result = sum_i cumsum(sorted_loss)[i-1]/i / B
Rewritten rank-wise: result = sum_j loss_j * w[rank_j] where
  rank_j = #{m : score_m > score_j},  score = exp(max x) / sum(exp x)
  w[r] = (H_{B-1} - H_r) / B,  H_r = sum_{i=1}^r 1/i  (asymptotic eval)

Sharding: batch B=8192 split 1024 rows/core. Each core streams its
[1024, 32000] shard once (row max via DVE reduce, exp-sum via the ACT
accumulator into a dead bf16 sink, target logit via indirect DMA mid-
stream). Ranking keys are the raw scores e^m/S (same order as the
log-domain key, but needs no mid-stream Ln -> no ACT table switches).
The 8192 score scalars are AllGathered in three pieces (groups 0-3,
4-6, 7) so only the last 512 B gather sits in the tail. Rank counts
run per group on one engine (DVE is_gt for groups {0-3,7}, ACT sign
for {4-6}), interleaved into stream slack against PE-replicated score
rows. The tail evaluates w(rank) analytically, takes one Ln table
switch for the losses, and emits a per-core partial dot product that
the host sums.
"""
import sys

if "/opt/trn_rl_repo" not in sys.path:
    sys.path.insert(0, "/opt/trn_rl_repo")

import numpy as np

B, C = 8192, 32000
NCORES = 8
BL = B // NCORES          # rows per core
P = 128                   # partitions
NG = BL // P              # row-groups per core
CF = 8000                 # columns per streamed chunk
# AllGather pieces: groups [0,4), [4,7), [7,8)
PIECES = [(0, 4), (4, 7), (7, 8)]
PW = [(hi - lo) * P * NCORES for lo, hi in PIECES]   # 4096, 3072, 1024
POFF = [0, PW[0], PW[0] + PW[1]]
ACT_GROUPS = (4, 5, 6)    # sign-counted on ACT; others is_gt on DVE
# tapered chunks for the last group so DVE/ACT drain with the DMA
LAST_BOUNDS = [0, 8000, 16000, 22400, 26400, 28800, 30400, 32000]

_CACHE = {}


def _build(debug: bool = False):
    import concourse.bass as bass
    import concourse.bacc as bacc
    import concourse.mybir as mybir
    import concourse.tile as tile

    nc = bacc.Bacc(num_devices=NCORES)
    x = nc.dram_tensor("x", [BL, C], mybir.dt.float32, kind="ExternalInput")
    # flat element offsets r*C + tgt[r], laid out so tile[p, g] = row g*P+p
    toff = nc.dram_tensor("toff", [BL], mybir.dt.int32, kind="ExternalInput")
    out = nc.dram_tensor("out", [1, 1], mybir.dt.float32, kind="ExternalOutput")

    xflat = x.rearrange("a b -> (a b)").unsqueeze(1)
    f32 = mybir.dt.float32
    bf16 = mybir.dt.bfloat16
    i32 = mybir.dt.int32
    AX = mybir.AxisListType.X
    OP = mybir.AluOpType
    AF = mybir.ActivationFunctionType

    with tile.TileContext(nc) as tc:
        with (
            tc.tile_pool(name="sb", bufs=3) as sb,
            tc.tile_pool(name="sm", bufs=1) as sm,
            tc.tile_pool(name="ps", bufs=1, space="PSUM") as ps,
            tc.tile_pool(name="dr", bufs=1, space="DRAM") as dr,
        ):
            ones_col = sm.tile([P, 1], f32)
            nc.vector.memset(ones_col[:, :], 1.0)
            ones_row = sm.tile([1, P], f32)
            nc.vector.memset(ones_row[:, :], 1.0)

            m_all = sm.tile([P, NG], f32)
            s_all = sm.tile([P, NG], f32)
            em = sm.tile([P, NG], f32)
            rinv = sm.tile([P, NG], f32)
            score = sm.tile([P, NG], f32)
            negs = sm.tile([P, NG], f32)
            lns = sm.tile([P, NG], f32)
            losses = sm.tile([P, NG], f32)
            xt = sm.tile([P, NG], f32)
            off_t = sm.tile([P, NG], i32)
            ranks = sm.tile([P, NG], f32)
            rparts = sm.tile([P, NG, 3], f32)
            # replicated global scores: piece p at cols [POFF[p], +PW[p])
            greph = sm.tile([P, B], f32)
            kl_in = [dr.tile([PW[i] // NCORES], f32, name=f"kl_in{i}")
                     for i in range(3)]
            kl_all = [dr.tile([PW[i]], f32, name=f"kl_all{i}")
                      for i in range(3)]
            # one staging row reused by all three pieces (disjoint in time)
            kl_row = sm.tile([1, PW[0]], f32)

            def stream_part(g, mx, sms, bounds, ci0, ci1):
                for c in range(ci0, ci1):
                    lo, hi = bounds[c], bounds[c + 1]
                    t = sb.tile([P, CF], f32, tag="t", name=f"t{g}_{c}")
                    nc.sync.dma_start(
                        t[:, :hi - lo], x[g * P:(g + 1) * P, lo:hi])
                    nc.vector.reduce_max(mx[:, c:c + 1], t[:, :hi - lo],
                                         axis=AX)
                    es = sb.tile([P, CF], bf16, tag="es", bufs=1,
                                 name=f"es{g}_{c}")
                    nc.scalar.activation(out=es[:, :hi - lo],
                                         in_=t[:, :hi - lo], func=AF.Exp,
                                         accum_out=sms[:, c:c + 1])

            def make_stats(g):
                nck = len(LAST_BOUNDS) - 1 if g == NG - 1 else C // CF
                tag = "mxL" if g == NG - 1 else "mx"
                mx = sm.tile([P, nck], f32, tag=tag, bufs=3, name=f"mx{g}")
                sms = sm.tile([P, nck], f32, tag=tag + "s", bufs=3,
                              name=f"sms{g}")
                return mx, sms

            def epilogue(g, mx, sms):
                nc.vector.reduce_max(m_all[:, g:g + 1], mx[:, :], axis=AX)
                nc.vector.reduce_sum(s_all[:, g:g + 1], sms[:, :], axis=AX)
                nc.vector.reciprocal(rinv[:, g:g + 1], s_all[:, g:g + 1])
                nc.scalar.activation(out=em[:, g:g + 1], in_=m_all[:, g:g + 1],
                                     func=AF.Exp)
                nc.vector.tensor_tensor(out=score[:, g:g + 1],
                                        in0=em[:, g:g + 1],
                                        in1=rinv[:, g:g + 1], op=OP.mult)
                if g in ACT_GROUPS:
                    nc.vector.tensor_scalar_mul(negs[:, g:g + 1],
                                                score[:, g:g + 1], -1.0)

            def stream_group(g):
                mx, sms = make_stats(g)
                bounds = [c * CF for c in range(C // CF)] + [C]
                stream_part(g, mx, sms, bounds, 0, len(bounds) - 1)
                epilogue(g, mx, sms)

            def ag(piece):
                glo, ghi = PIECES[piece]
                nc.scalar.dma_start(
                    kl_in[piece][:].rearrange("(p g) -> p g", g=ghi - glo),
                    score[:, glo:ghi])
                nc.gpsimd.collective_compute(
                    "AllGather", OP.bypass,
                    replica_groups=[list(range(NCORES))],
                    ins=[kl_in[piece].opt()], outs=[kl_all[piece].opt()])

            def fetch_row(piece):
                nc.gpsimd.dma_start(kl_row[:, :PW[piece]],
                                    kl_all[piece][:].unsqueeze(0))

            def replicate(piece, jlo, jhi, copy_eng):
                # broadcast kl_row[piece] cols [jlo, jhi) to all partitions
                # of greph via PE ones-matmul (keeps HBM free)
                for k, j in enumerate(range(jlo, jhi, 512)):
                    pt = ps.tile([P, 512], f32, tag="pt", bufs=2,
                                 name=f"pt{piece}_{j}")
                    nc.tensor.matmul(pt[:, :], lhsT=ones_row[:, :],
                                     rhs=kl_row[:, j:j + 512],
                                     start=True, stop=True)
                    dst = greph[:, POFF[piece] + j:POFF[piece] + j + 512]
                    eng = copy_eng[k % len(copy_eng)]
                    if eng == "act":
                        nc.scalar.copy(dst, pt[:, :])
                    else:
                        nc.vector.tensor_copy(dst, pt[:, :])

            def cmp(g, piece):
                # count global scores above score_g within one piece
                lo, w = POFF[piece], PW[piece]
                acc = rparts[:, g, piece:piece + 1]
                src = greph[:, lo:lo + w]
                if g in ACT_GROUPS:
                    s = sb.tile([P, PW[0]], bf16, tag="cas", bufs=1,
                                name=f"ca{g}_{piece}")
                    nc.scalar.activation(out=s[:, :w], in_=src,
                                         func=AF.Sign,
                                         bias=negs[:, g:g + 1],
                                         accum_out=acc)
                else:
                    s = sb.tile([P, PW[0]], bf16, tag="cds", bufs=1,
                                name=f"cd{g}_{piece}")
                    nc.vector.tensor_scalar(
                        out=s[:, :w], in0=src, scalar1=score[:, g:g + 1],
                        scalar2=None, op0=OP.is_gt, op1=OP.add,
                        accum_out=acc)

            # ---- stream groups 0-3; AllGather their scores ----
            for g in range(4):
                stream_group(g)
            ag(0)

            stream_group(4)
            # target-logit gather tucked mid-stream (needed only at the end)
            nc.gpsimd.dma_start(off_t[:, :],
                                toff.rearrange("(p g) -> p g", g=NG))
            for g in range(NG):
                nc.gpsimd.indirect_dma_start(
                    out=xt[:, g:g + 1], out_offset=None, in_=xflat,
                    in_offset=bass.IndirectOffsetOnAxis(ap=off_t[:, g:g + 1],
                                                        axis=0))
            fetch_row(0)

            stream_group(5)
            replicate(0, 0, PW[0], ("dve", "act"))
            for g in (0, 1, 2):
                cmp(g, 0)

            mx6, sms6 = make_stats(6)
            b6 = [c * CF for c in range(C // CF)] + [C]
            stream_part(6, mx6, sms6, b6, 0, 4)
            cmp(3, 0)
            cmp(4, 0)
            cmp(5, 0)
            epilogue(6, mx6, sms6)
            ag(1)
            cmp(6, 0)

            mx7, sms7 = make_stats(7)
            stream_part(7, mx7, sms7, LAST_BOUNDS, 0, 3)
            fetch_row(1)
            replicate(1, 0, PW[1], ("act", "dve"))
            for g in (0, 1):
                cmp(g, 1)
            for g in (4, 5):
                cmp(g, 1)
            stream_part(7, mx7, sms7, LAST_BOUNDS, 3, 7)
            for g in (2, 3):
                cmp(g, 1)
            cmp(6, 1)
            epilogue(7, mx7, sms7)

            # ---- tail: last 128 scores AllGathered and compared ----
            ag(2)
            cmp(7, 0)
            cmp(7, 1)
            # losses need ln(S); one table switch, hidden under the gather
            nc.scalar.activation(out=lns[:, :], in_=s_all[:, :], func=AF.Ln)
            nc.vector.tensor_tensor(out=losses[:, :], in0=lns[:, :],
                                    in1=xt[:, :], op=OP.subtract)
            fetch_row(2)
            replicate(2, 0, PW[2], ("act", "dve"))
            for g in range(NG):
                cmp(g, 2)
            for g in range(NG):
                nc.vector.reduce_sum(ranks[:, g:g + 1], rparts[:, g, :],
                                     axis=AX)
            # sign-counted groups: rank = (cnt + B-1)/2
            a0, a1 = ACT_GROUPS[0], ACT_GROUPS[-1] + 1
            nc.vector.tensor_scalar(
                out=ranks[:, a0:a1], in0=ranks[:, a0:a1],
                scalar1=0.5, scalar2=float((B - 1) / 2),
                op0=OP.mult, op1=OP.add)

            # w(rank) analytically: H_r = ln(r+1) + g - u(0.5 + u/12),
            # u = 1/(r+1);  w = (H_{B-1}-g)/B - (ln(r+1) - u(0.5+u/12))/B
            EUL = 0.5772156649015329
            h_top = float(np.sum(1.0 / np.arange(1, B, dtype=np.float64)))
            C0 = float((h_top - EUL) / B)
            tt = sm.tile([P, NG], f32)
            nc.vector.tensor_scalar_add(tt[:, :], ranks[:, :], 1.0)
            lnt = sm.tile([P, NG], f32)
            nc.scalar.activation(out=lnt[:, :], in_=tt[:, :], func=AF.Ln)
            u = sm.tile([P, NG], f32)
            nc.vector.reciprocal(u[:, :], tt[:, :])
            v = sm.tile([P, NG], f32)
            nc.vector.tensor_scalar(out=v[:, :], in0=u[:, :],
                                    scalar1=float(1 / 12), scalar2=0.5,
                                    op0=OP.mult, op1=OP.add)
            nc.vector.tensor_tensor(out=v[:, :], in0=v[:, :], in1=u[:, :],
                                    op=OP.mult)
            nc.vector.tensor_tensor(out=v[:, :], in0=lnt[:, :], in1=v[:, :],
                                    op=OP.subtract)
            wg = sm.tile([P, NG], f32)
            nc.vector.tensor_scalar(out=wg[:, :], in0=v[:, :],
                                    scalar1=float(-1.0 / B), scalar2=C0,
                                    op0=OP.mult, op1=OP.add)
            prod = sm.tile([P, NG], f32)
            nc.vector.tensor_tensor(out=prod[:, :], in0=wg[:, :],
                                    in1=losses[:, :], op=OP.mult)

            prow = sm.tile([P, 1], f32)
            nc.vector.reduce_sum(prow[:, :], prod[:, :], axis=AX)
            pscal = ps.tile([1, 1], f32, tag="pscal")
            nc.tensor.matmul(pscal[:, :], lhsT=prow[:, :], rhs=ones_col[:, :],
                             start=True, stop=True)
            psb = sm.tile([1, 1], f32)
            nc.scalar.copy(psb[:, :], pscal[:, :])
            nc.sync.dma_start(out[:, :], psb[:, :])

            if debug:
                for nm, tl in [("dkeys", score), ("dloss", losses),
                               ("dranks", ranks), ("dwg", wg), ("dxt", xt)]:
                    dt_ = nc.dram_tensor(nm, list(tl.shape), f32,
                                         kind="ExternalOutput")
                    nc.sync.dma_start(dt_[:, :], tl[:, :])

    nc.finalize()
    return nc


def _shard_inputs(input: np.ndarray, target: np.ndarray):
    xin = np.ascontiguousarray(input, dtype=np.float32)
    toff = (np.arange(B, dtype=np.int64) % BL) * C + target.astype(np.int64)
    toff = toff.astype(np.int32).reshape(NCORES, NG, P)
    # tile[p, g] = row g*P+p  ->  flat host order (p, g)
    toff = np.ascontiguousarray(toff.transpose(0, 2, 1)).reshape(NCORES, BL)
    return [
        {"x": xin[c * BL:(c + 1) * BL], "toff": toff[c]}
        for c in range(NCORES)
    ]


def _run(input: np.ndarray, target: np.ndarray, trace: bool = False):
    from concourse.bass_utils import run_bass_kernel_spmd

    if "nc" not in _CACHE:
        _CACHE["nc"] = _build()
    nc = _CACHE["nc"]

    in_maps = _shard_inputs(input, target)
    res = run_bass_kernel_spmd(nc, in_maps, core_ids=list(range(NCORES)),
                               trace=trace)
    parts = [r["out"][0, 0] for r in res.results]
    total = np.float32(np.sum(np.asarray(parts, dtype=np.float64)))
    return np.asarray(total, dtype=np.float32), res


def kernel(input: np.ndarray, target: np.ndarray) -> np.ndarray:
    out, _ = _run(input, target, trace=False)
    return out


# revision 13
# speedup vs baseline: 1.0822x; 1.0822x over previous
"""AURC loss kernel for Trainium2, sharded across 8 NeuronCores.

Algorithm (matches the reference):
  logp = log_softmax(x);  score = exp(max logp);  loss = -logp[target]
  sort by score desc;  result = sum_i cumsum(sorted_loss)[i-1]/i / B
Rewritten rank-wise: result = sum_j loss_j * w[rank_j] where
  rank_j = #{m : score_m > score_j},  score = exp(max x) / sum(exp x)
  w[r] = (H_{B-1} - H_r) / B,  H_r = sum_{i=1}^r 1/i  (asymptotic eval)

Sharding: batch B=8192 split 1024 rows/core. Each core streams its
[1024, 32000] shard once (row max via DVE reduce, exp-sum via the ACT
accumulator into a dead bf16 sink, target logit via indirect DMA mid-
stream). Ranking keys are the raw scores e^m/S (same order as the
log-domain key, but needs no mid-stream Ln -> no ACT table switches).
The 8192 score scalars are AllGathered in three pieces (groups 0-3,
4-6, 7) so only the last 512 B gather sits in the tail. Rank counts
run per group on one engine (DVE is_gt for groups {0-3,7}, ACT sign
for {4-6}), interleaved into stream slack against PE-replicated score
rows. The tail evaluates w(rank) analytically, takes one Ln table
switch for the losses, and emits a per-core partial dot product that
the host sums.
"""
import sys

if "/opt/trn_rl_repo" not in sys.path:
    sys.path.insert(0, "/opt/trn_rl_repo")

import numpy as np

B, C = 8192, 32000
NCORES = 8
BL = B // NCORES          # rows per core
P = 128                   # partitions
NG = BL // P              # row-groups per core
CF = 8000                 # columns per streamed chunk
# AllGather pieces: groups [0,4), [4,7), [7,8)
PIECES = [(0, 4), (4, 7), (7, 8)]
PW = [(hi - lo) * P * NCORES for lo, hi in PIECES]   # 4096, 3072, 1024
POFF = [0, PW[0], PW[0] + PW[1]]
ACT_GROUPS = (2, 3, 4, 5, 6)   # sign-counted on ACT; {0,1,7} is_gt on DVE
# tapered chunks for the last group so DVE/ACT drain with the DMA
LAST_BOUNDS = [0, 8000, 16000, 22400, 26400, 28800, 30400, 32000]

_CACHE = {}


def _build(debug: bool = False):
    import concourse.bass as bass
    import concourse.bacc as bacc
    import concourse.mybir as mybir
    import concourse.tile as tile

    nc = bacc.Bacc(num_devices=NCORES)
    x = nc.dram_tensor("x", [BL, C], mybir.dt.float32, kind="ExternalInput")
    # flat element offsets r*C + tgt[r], laid out so tile[p, g] = row g*P+p
    toff = nc.dram_tensor("toff", [BL], mybir.dt.int32, kind="ExternalInput")
    out = nc.dram_tensor("out", [1, 1], mybir.dt.float32, kind="ExternalOutput")

    xflat = x.rearrange("a b -> (a b)").unsqueeze(1)
    f32 = mybir.dt.float32
    bf16 = mybir.dt.bfloat16
    i32 = mybir.dt.int32
    AX = mybir.AxisListType.X
    OP = mybir.AluOpType
    AF = mybir.ActivationFunctionType

    with tile.TileContext(nc) as tc:
        with (
            tc.tile_pool(name="sb", bufs=3) as sb,
            tc.tile_pool(name="sm", bufs=1) as sm,
            tc.tile_pool(name="ps", bufs=1, space="PSUM") as ps,
            tc.tile_pool(name="dr", bufs=1, space="DRAM") as dr,
        ):
            ones_col = sm.tile([P, 1], f32)
            nc.vector.memset(ones_col[:, :], 1.0)
            ones_row = sm.tile([1, P], f32)
            nc.vector.memset(ones_row[:, :], 1.0)

            m_all = sm.tile([P, NG], f32)
            s_all = sm.tile([P, NG], f32)
            em = sm.tile([P, NG], f32)
            rinv = sm.tile([P, NG], f32)
            score = sm.tile([P, NG], f32)
            negs = sm.tile([P, NG], f32)
            lns = sm.tile([P, NG], f32)
            losses = sm.tile([P, NG], f32)
            xt = sm.tile([P, NG], f32)
            off_t = sm.tile([P, NG], i32)
            ranks = sm.tile([P, NG], f32)
            s01 = sm.tile([P, NG], f32)
            rparts = sm.tile([P, 3, NG], f32)
            # replicated global scores: piece p at cols [POFF[p], +PW[p])
            greph = sm.tile([P, B], f32)
            kl_in = [dr.tile([PW[i] // NCORES], f32, name=f"kl_in{i}")
                     for i in range(3)]
            kl_all = [dr.tile([PW[i]], f32, name=f"kl_all{i}")
                      for i in range(3)]
            # one staging row reused by all three pieces (disjoint in time)
            kl_row = sm.tile([1, PW[0]], f32)

            def stream_part(g, mx, sms, bounds, ci0, ci1):
                for c in range(ci0, ci1):
                    lo, hi = bounds[c], bounds[c + 1]
                    t = sb.tile([P, CF], f32, tag="t", name=f"t{g}_{c}")
                    nc.sync.dma_start(
                        t[:, :hi - lo], x[g * P:(g + 1) * P, lo:hi])
                    nc.vector.reduce_max(mx[:, c:c + 1], t[:, :hi - lo],
                                         axis=AX)
                    # in place: serializes MAX before EXP on this tile, which
                    # avoids two engines reading the same SBUF lines at once
                    nc.scalar.activation(out=t[:, :hi - lo],
                                         in_=t[:, :hi - lo], func=AF.Exp,
                                         accum_out=sms[:, c:c + 1])

            def make_stats(g):
                nck = len(LAST_BOUNDS) - 1 if g == NG - 1 else C // CF
                tag = "mxL" if g == NG - 1 else "mx"
                mx = sm.tile([P, nck], f32, tag=tag, bufs=3, name=f"mx{g}")
                sms = sm.tile([P, nck], f32, tag=tag + "s", bufs=3,
                              name=f"sms{g}")
                return mx, sms

            def epilogue(g, mx, sms):
                nc.vector.reduce_max(m_all[:, g:g + 1], mx[:, :], axis=AX)
                nc.vector.reduce_sum(s_all[:, g:g + 1], sms[:, :], axis=AX)
                nc.vector.reciprocal(rinv[:, g:g + 1], s_all[:, g:g + 1])
                nc.scalar.activation(out=em[:, g:g + 1], in_=m_all[:, g:g + 1],
                                     func=AF.Exp)
                nc.vector.tensor_tensor(out=score[:, g:g + 1],
                                        in0=em[:, g:g + 1],
                                        in1=rinv[:, g:g + 1], op=OP.mult)
                if g in ACT_GROUPS:
                    nc.vector.tensor_scalar_mul(negs[:, g:g + 1],
                                                score[:, g:g + 1], -1.0)

            def stream_group(g):
                mx, sms = make_stats(g)
                bounds = [c * CF for c in range(C // CF)] + [C]
                stream_part(g, mx, sms, bounds, 0, len(bounds) - 1)
                epilogue(g, mx, sms)

            def ag(piece):
                glo, ghi = PIECES[piece]
                nc.scalar.dma_start(
                    kl_in[piece][:].rearrange("(p g) -> p g", g=ghi - glo),
                    score[:, glo:ghi])
                nc.gpsimd.collective_compute(
                    "AllGather", OP.bypass,
                    replica_groups=[list(range(NCORES))],
                    ins=[kl_in[piece].opt()], outs=[kl_all[piece].opt()])

            def fetch_row(piece):
                nc.gpsimd.dma_start(kl_row[:, :PW[piece]],
                                    kl_all[piece][:].unsqueeze(0))

            def replicate(piece, jlo, jhi, copy_eng):
                # broadcast kl_row[piece] cols [jlo, jhi) to all partitions
                # of greph via PE ones-matmul (keeps HBM free)
                for k, j in enumerate(range(jlo, jhi, 512)):
                    pt = ps.tile([P, 512], f32, tag="pt", bufs=2,
                                 name=f"pt{piece}_{j}")
                    nc.tensor.matmul(pt[:, :], lhsT=ones_row[:, :],
                                     rhs=kl_row[:, j:j + 512],
                                     start=True, stop=True)
                    dst = greph[:, POFF[piece] + j:POFF[piece] + j + 512]
                    eng = copy_eng[k % len(copy_eng)]
                    if eng == "act":
                        nc.scalar.copy(dst, pt[:, :])
                    else:
                        nc.vector.tensor_copy(dst, pt[:, :])

            def cmp(g, piece):
                # count global scores above score_g within one piece
                lo, w = POFF[piece], PW[piece]
                acc = rparts[:, piece, g:g + 1]
                src = greph[:, lo:lo + w]
                if g in ACT_GROUPS:
                    s = sb.tile([P, PW[0]], bf16, tag="cas", bufs=1,
                                name=f"ca{g}_{piece}")
                    nc.scalar.activation(out=s[:, :w], in_=src,
                                         func=AF.Sign,
                                         bias=negs[:, g:g + 1],
                                         accum_out=acc)
                else:
                    s = sb.tile([P, PW[0]], bf16, tag="cds", bufs=1,
                                name=f"cd{g}_{piece}")
                    nc.vector.tensor_scalar(
                        out=s[:, :w], in0=src, scalar1=score[:, g:g + 1],
                        scalar2=None, op0=OP.is_gt, op1=OP.add,
                        accum_out=acc)

            # ---- stream groups 0-3; AllGather their scores ----
            for g in range(4):
                stream_group(g)
            ag(0)

            stream_group(4)
            # target-logit gather tucked mid-stream (needed only at the end)
            nc.gpsimd.dma_start(off_t[:, :],
                                toff.rearrange("(p g) -> p g", g=NG))
            for g in range(NG):
                nc.gpsimd.indirect_dma_start(
                    out=xt[:, g:g + 1], out_offset=None, in_=xflat,
                    in_offset=bass.IndirectOffsetOnAxis(ap=off_t[:, g:g + 1],
                                                        axis=0))
            fetch_row(0)

            stream_group(5)
            replicate(0, 0, PW[0], ("act", "dve"))
            for g in (0, 1):          # DVE groups vs piece 0
                cmp(g, 0)
            for g in (2, 3):          # ACT groups vs piece 0
                cmp(g, 0)

            mx6, sms6 = make_stats(6)
            b6 = [c * CF for c in range(C // CF)] + [C]
            stream_part(6, mx6, sms6, b6, 0, 4)
            cmp(4, 0)
            cmp(5, 0)
            epilogue(6, mx6, sms6)
            ag(1)
            cmp(6, 0)

            mx7, sms7 = make_stats(7)
            stream_part(7, mx7, sms7, LAST_BOUNDS, 0, 3)
            fetch_row(1)
            replicate(1, 0, PW[1], ("act", "dve"))
            for g in (0, 1):          # DVE groups vs piece 1
                cmp(g, 1)
            for g in (2, 3):
                cmp(g, 1)
            stream_part(7, mx7, sms7, LAST_BOUNDS, 3, 7)
            cmp(4, 1)
            epilogue(7, mx7, sms7)

            # ---- tail: last 128 scores AllGathered and compared ----
            ag(2)
            # work that hides under the gather's flight:
            cmp(7, 0)                 # DVE
            cmp(7, 1)                 # DVE
            cmp(5, 1)                 # ACT
            cmp(6, 1)                 # ACT
            # losses need ln(S); one table switch, hidden under the gather
            nc.scalar.activation(out=lns[:, :], in_=s_all[:, :], func=AF.Ln)
            nc.vector.tensor_tensor(out=losses[:, :], in0=lns[:, :],
                                    in1=xt[:, :], op=OP.subtract)
            nc.vector.tensor_tensor(out=s01[:, :], in0=rparts[:, 0, :],
                                    in1=rparts[:, 1, :], op=OP.add)
            nc.scalar.dma_start(kl_row[:, :PW[2]],
                                kl_all[2][:].unsqueeze(0))
            replicate(2, 0, PW[2], ("act", "dve"))
            for g in (0, 1, 7, 2, 3, 4, 5, 6):
                cmp(g, 2)
            nc.vector.tensor_tensor(out=ranks[:, :], in0=s01[:, :],
                                    in1=rparts[:, 2, :], op=OP.add)
            # sign-counted groups: rank = (cnt + B-1)/2
            a0, a1 = ACT_GROUPS[0], ACT_GROUPS[-1] + 1
            nc.vector.tensor_scalar(
                out=ranks[:, a0:a1], in0=ranks[:, a0:a1],
                scalar1=0.5, scalar2=float((B - 1) / 2),
                op0=OP.mult, op1=OP.add)

            # w(rank) analytically: H_r = ln(r+1) + g - u(0.5 + u/12),
            # u = 1/(r+1);  w = (H_{B-1}-g)/B - (ln(r+1) - u(0.5+u/12))/B
            EUL = 0.5772156649015329
            h_top = float(np.sum(1.0 / np.arange(1, B, dtype=np.float64)))
            C0 = float((h_top - EUL) / B)
            tt = sm.tile([P, NG], f32)
            nc.vector.tensor_scalar_add(tt[:, :], ranks[:, :], 1.0)
            lnt = sm.tile([P, NG], f32)
            nc.scalar.activation(out=lnt[:, :], in_=tt[:, :], func=AF.Ln)
            u = sm.tile([P, NG], f32)
            nc.vector.reciprocal(u[:, :], tt[:, :])
            v = sm.tile([P, NG], f32)
            nc.vector.tensor_scalar(out=v[:, :], in0=u[:, :],
                                    scalar1=float(1 / 12), scalar2=0.5,
                                    op0=OP.mult, op1=OP.add)
            nc.vector.tensor_tensor(out=v[:, :], in0=v[:, :], in1=u[:, :],
                                    op=OP.mult)
            nc.vector.tensor_tensor(out=v[:, :], in0=lnt[:, :], in1=v[:, :],
                                    op=OP.subtract)
            wg = sm.tile([P, NG], f32)
            nc.vector.tensor_scalar(out=wg[:, :], in0=v[:, :],
                                    scalar1=float(-1.0 / B), scalar2=C0,
                                    op0=OP.mult, op1=OP.add)
            prod = sm.tile([P, NG], f32)
            nc.vector.tensor_tensor(out=prod[:, :], in0=wg[:, :],
                                    in1=losses[:, :], op=OP.mult)

            prow = sm.tile([P, 1], f32)
            nc.vector.reduce_sum(prow[:, :], prod[:, :], axis=AX)
            pscal = ps.tile([1, 1], f32, tag="pscal")
            nc.tensor.matmul(pscal[:, :], lhsT=prow[:, :], rhs=ones_col[:, :],
                             start=True, stop=True)
            psb = sm.tile([1, 1], f32)
            nc.scalar.copy(psb[:, :], pscal[:, :])
            nc.sync.dma_start(out[:, :], psb[:, :])

            if debug:
                for nm, tl in [("dkeys", score), ("dloss", losses),
                               ("dranks", ranks), ("dwg", wg), ("dxt", xt)]:
                    dt_ = nc.dram_tensor(nm, list(tl.shape), f32,
                                         kind="ExternalOutput")
                    nc.sync.dma_start(dt_[:, :], tl[:, :])

    nc.finalize()
    return nc


def _shard_inputs(input: np.ndarray, target: np.ndarray):
    xin = np.ascontiguousarray(input, dtype=np.float32)
    toff = (np.arange(B, dtype=np.int64) % BL) * C + target.astype(np.int64)
    toff = toff.astype(np.int32).reshape(NCORES, NG, P)
    # tile[p, g] = row g*P+p  ->  flat host order (p, g)
    toff = np.ascontiguousarray(toff.transpose(0, 2, 1)).reshape(NCORES, BL)
    return [
        {"x": xin[c * BL:(c + 1) * BL], "toff": toff[c]}
        for c in range(NCORES)
    ]


def _run(input: np.ndarray, target: np.ndarray, trace: bool = False):
    from concourse.bass_utils import run_bass_kernel_spmd

    if "nc" not in _CACHE:
        _CACHE["nc"] = _build()
    nc = _CACHE["nc"]

    in_maps = _shard_inputs(input, target)
    res = run_bass_kernel_spmd(nc, in_maps, core_ids=list(range(NCORES)),
                               trace=trace)
    parts = [r["out"][0, 0] for r in res.results]
    total = np.float32(np.sum(np.asarray(parts, dtype=np.float64)))
    return np.asarray(total, dtype=np.float32), res


def kernel(input: np.ndarray, target: np.ndarray) -> np.ndarray:
    out, _ = _run(input, target, trace=False)
    return out
